# revision 12
# baseline (speedup 1.0000x reference)
"""Trainium2 Bass kernel for nn_BAFM_BRB_65249143161427 (segment_reduce).

Reference semantics: downsample x0/x1 by 8 (nearest), compute directional
running segment means between zero-boundaries of x1 along rows and columns,
sum the 4 directional terms, upsample by 8.

Sharding: pure data parallel — image n -> core n (N=8, 8 cores).
Each core processes one full 2048x2048 image.
"""
import sys

sys.path.insert(0, "/opt/trn_rl_repo")

import numpy as np

H = W = 2048
S = 8
HD, WD = H // S, W // S      # 256 x 256 downsampled grid
P = 128                      # SBUF partitions
NT = HD // P                 # 2 row-tiles of the downsampled grid
N_CORES = 8

_CACHE = {}


def _revap(ap2d):
    """Reverse the last (free) dim of a 2D access pattern."""
    from concourse.ap import AP

    pairs = [list(p) for p in ap2d.ap]
    step, n = pairs[-1]
    return AP(ap2d.tensor, ap2d.offset + (n - 1) * step, pairs[:-1] + [[-step, n]])


def _bcast8(ap2d):
    """Append a step-0 count-8 inner dim (each element replicated 8x)."""
    from concourse.ap import AP

    pairs = [list(p) for p in ap2d.ap]
    return AP(ap2d.tensor, ap2d.offset, pairs + [[0, 8]])


def build_program(loop_n: int = 1):
    import concourse.bacc as bacc
    import concourse.tile as tile
    from concourse import mybir
    from concourse.masks import make_identity
    from contextlib import ExitStack, nullcontext

    f32 = mybir.dt.float32
    i32 = mybir.dt.int32
    MUL = mybir.AluOpType.mult
    ADD = mybir.AluOpType.add
    NE = mybir.AluOpType.not_equal

    # Bacc (not raw Bass): its compile() splits multi-wait sync commands,
    # which TRN2 engines (1 wait/instruction) require.
    nc = bacc.Bacc("TRN2")
    x0 = nc.declare_dram_parameter("x0", [H, W], f32, isOutput=False)
    x1 = nc.declare_dram_parameter("x1", [H, W], i32, isOutput=False)
    y = nc.declare_dram_parameter("y", [H, W], f32, isOutput=True)

    # DRAM viewed with rows grouped by 8: [256, 8, 2048]
    x0g = x0[:].rearrange("(r e) w -> r e w", e=8)
    x1g = x1[:].rearrange("(r e) w -> r e w", e=8)
    yg = y[:].rearrange("(r e) w -> r e w", e=8)

    with tile.TileContext(nc) as tc:
        with ExitStack() as ctx:
            const_pool = ctx.enter_context(tc.tile_pool(name="const", bufs=1))
            io_pool = ctx.enter_context(tc.tile_pool(name="io", bufs=1))
            work = ctx.enter_context(tc.tile_pool(name="work", bufs=1))
            psum = ctx.enter_context(
                tc.tile_pool(name="psum", bufs=4, space="PSUM")
            )

            ident_g = const_pool.tile([P, P], f32)
            make_identity(nc, ident_g[:])
            ident = const_pool.tile([P, P], f32)
            nc.vector.tensor_copy(ident[:], ident_g[:])
            ones = const_pool.tile([P, WD], f32)
            nc.gpsimd.memset(ones[:], 1.0)

            def body():
                # ---- load every 8th row, downsample cols, build mask ----
                xd = []   # downsampled values, [128, 256] x2
                am = []   # float mask: 1.0 where x1 != 0 (not a boundary)
                for t in range(NT):
                    x0r = io_pool.tile([P, W], f32, tag=f"x0r{t}")
                    x1r = io_pool.tile([P, W], i32, tag=f"x1r{t}")
                    nc.sync.dma_start(
                        out=x0r[:], in_=x0g[t * P:(t + 1) * P, 0, :]
                    )
                    nc.sync.dma_start(
                        out=x1r[:], in_=x1g[t * P:(t + 1) * P, 0, :]
                    )
                    xdt = work.tile([P, WD], f32, tag=f"xd{t}")
                    amt = work.tile([P, WD], f32, tag=f"am{t}")
                    nc.vector.tensor_copy(
                        xdt[:], x0r[:].rearrange("p (a b) -> p a b", b=8)[:, :, 0]
                    )
                    nc.vector.tensor_scalar(
                        out=amt[:],
                        in0=x1r[:].rearrange("p (a b) -> p a b", b=8)[:, :, 0],
                        scalar1=0, scalar2=None, op0=NE,
                    )
                    xd.append(xdt)
                    am.append(amt)

                # ---- directional segment-mean pass over a list of tiles ----
                def seg_pass(xts, ats, name):
                    """m[t] = where(q==0, lr+rl, 2*x) for each input tile."""
                    outs = []
                    for t, (xt, at) in enumerate(zip(xts, ats)):
                        pf = f"{name}{t}"
                        s_lr = work.tile([P, WD], f32, tag=f"slr{pf}")
                        c_lr = work.tile([P, WD], f32, tag=f"clr{pf}")
                        nb = work.tile([P, WD], f32, tag=f"nb{pf}")
                        s_rl = work.tile([P, WD], f32, tag=f"srl{pf}")  # rev
                        c_rl = work.tile([P, WD], f32, tag=f"crl{pf}")  # rev
                        na = work.tile([P, WD], f32, tag=f"na{pf}")     # rev
                        a_r = _revap(at[:])
                        x_r = _revap(xt[:])
                        ttscan = nc.vector.tensor_tensor_scan
                        ttscan(s_lr[:], at[:], xt[:], 0.0, MUL, ADD)
                        ttscan(c_lr[:], at[:], ones[:], 0.0, MUL, ADD)
                        ttscan(nb[:], at[:], at[:], 1.0, MUL, MUL)
                        ttscan(s_rl[:], a_r, x_r, 0.0, MUL, ADD)
                        ttscan(c_rl[:], a_r, ones[:], 0.0, MUL, ADD)
                        ttscan(na[:], a_r, a_r, 1.0, MUL, MUL)
                        # reciprocals of counts (~2 ULP)
                        ilr = work.tile([P, WD], f32, tag=f"ilr{pf}")
                        irl = work.tile([P, WD], f32, tag=f"irl{pf}")
                        scr = work.tile([P, WD], f32, tag=f"scr{pf}")
                        nc.vector.reciprocal_approx_accurate(ilr[:], c_lr[:], scr[:])
                        nc.vector.reciprocal_approx_accurate(irl[:], c_rl[:], scr[:])
                        # means; m = lr + reverse(rl_rev)
                        nc.vector.tensor_tensor(s_lr[:], s_lr[:], ilr[:], MUL)
                        nc.vector.tensor_tensor(s_rl[:], s_rl[:], irl[:], MUL)
                        m = work.tile([P, WD], f32, tag=f"m{pf}")
                        nc.vector.tensor_tensor(m[:], s_lr[:], _revap(s_rl[:]), ADD)
                        # q = nb + reverse(na): nonzero -> no enclosing bounds
                        # (int32 out: BIR requires an integer predicate dtype)
                        q = work.tile([P, WD], i32, tag=f"q{pf}")
                        nc.vector.tensor_tensor(q[:], nb[:], _revap(na[:]), ADD)
                        two_x = work.tile([P, WD], f32, tag=f"tx{pf}")
                        nc.scalar.mul(two_x[:], xt[:], 2.0)
                        nc.vector.copy_predicated(m[:], q[:], two_x[:])
                        outs.append(m)
                    return outs

                # ---- horizontal pass ----
                yh = seg_pass(xd, am, "h")

                # ---- transpose xd and mask for the vertical pass ----
                xT, aT = [], []
                for c in range(NT):
                    xTt = work.tile([P, HD], f32, tag=f"xT{c}")
                    aTt = work.tile([P, HD], f32, tag=f"aT{c}")
                    for t in range(NT):
                        for src, dst in ((xd, xTt), (am, aTt)):
                            pb = psum.tile([P, P], f32, tag="ptr")
                            nc.tensor.transpose(
                                pb[:], src[t][:, c * P:(c + 1) * P], ident[:]
                            )
                            nc.scalar.copy(dst[:, t * P:(t + 1) * P], pb[:])
                    xT.append(xTt)
                    aT.append(aTt)

                # ---- vertical pass on transposed grid ----
                yvT = seg_pass(xT, aT, "v")

                # ---- transpose back, accumulate: ytot[t] = yh[t] + yvT^T ----
                for t in range(NT):
                    for c in range(NT):
                        pb = psum.tile([P, P], f32, tag="ptb")
                        nc.tensor.transpose(
                            pb[:], yvT[c][:, t * P:(t + 1) * P], ident[:]
                        )
                        nc.vector.tensor_tensor(
                            yh[t][:, c * P:(c + 1) * P],
                            yh[t][:, c * P:(c + 1) * P],
                            pb[:], ADD,
                        )

                # ---- upsample 8x8 and store ----
                for t in range(NT):
                    ye = io_pool.tile([P, W], f32, tag=f"ye{t}")
                    nc.scalar.copy(
                        ye[:].rearrange("p (a b) -> p a b", b=8),
                        _bcast8(yh[t][:]),
                    )
                    for k in range(8):
                        nc.sync.dma_start(
                            out=yg[t * P:(t + 1) * P, k, :], in_=ye[:]
                        )

            if loop_n > 1:
                with tc.For_i(0, loop_n, 1):
                    body()
            else:
                body()

    nc.compile()
    return nc


def _get_nc():
    if "nc" not in _CACHE:
        _CACHE["nc"] = build_program()
    return _CACHE["nc"]


def kernel(x0: np.ndarray, x1: np.ndarray) -> np.ndarray:
    from concourse.bass_utils import run_bass_kernel_spmd

    nc = _get_nc()
    n = x0.shape[0]
    in_maps = [
        {"x0": np.ascontiguousarray(x0[i, 0]),
         "x1": np.ascontiguousarray(x1[i, 0])}
        for i in range(n)
    ]
    res = run_bass_kernel_spmd(nc, in_maps, list(range(N_CORES)))
    out = np.stack([res.results[i]["y"] for i in range(n)])
    return out.reshape(n, 1, H, W).astype(np.float32)


# revision 23
# speedup vs baseline: 1.1463x; 1.1463x over previous
"""Trainium2 Bass kernel for nn_BAFM_BRB_65249143161427 (segment_reduce).

Reference semantics: downsample x0/x1 by 8 (nearest), compute directional
running segment means between zero-boundaries of x1 along rows and columns,
sum the 4 directional terms, upsample by 8.

Sharding: pure data parallel — image n -> core n (N=8, 8 cores).
Each core processes one full 2048x2048 image.

Structure (per core): the 16 MB output store stream dominates (~47 us at
360 GB/s), so the program is ordered to start storing as early as possible:
load -> h-pass(tile0) -> transposes -> v-pass(col0) -> store block (0,0),
with the remaining passes' DVE work hidden under the store stream.
"""
import sys

sys.path.insert(0, "/opt/trn_rl_repo")

import numpy as np

H = W = 2048
S = 8
HD, WD = H // S, W // S      # 256 x 256 downsampled grid
P = 128                      # SBUF partitions
NT = HD // P                 # 2 row-tiles of the downsampled grid
N_CORES = 8

_CACHE = {}


def _revap(ap2d):
    """Reverse the last (free) dim of a 2D access pattern."""
    from concourse.ap import AP

    pairs = [list(p) for p in ap2d.ap]
    step, n = pairs[-1]
    return AP(ap2d.tensor, ap2d.offset + (n - 1) * step, pairs[:-1] + [[-step, n]])


def _bcast8(ap2d):
    """Append a step-0 count-8 inner dim (each element replicated 8x)."""
    from concourse.ap import AP

    pairs = [list(p) for p in ap2d.ap]
    return AP(ap2d.tensor, ap2d.offset, pairs + [[0, 8]])


def build_program(loop_n: int = 1, fast_recip=True, pool_elem=True,
                  pool_pre=True):
    import concourse.bacc as bacc
    import concourse.tile as tile
    from concourse import mybir
    from concourse.masks import make_identity
    from contextlib import ExitStack

    f32 = mybir.dt.float32
    i32 = mybir.dt.int32
    MUL = mybir.AluOpType.mult
    ADD = mybir.AluOpType.add
    NE = mybir.AluOpType.not_equal

    # Bacc (not raw Bass): its compile() splits multi-wait sync commands,
    # which TRN2 engines (1 wait/instruction) require.
    nc = bacc.Bacc("TRN2")
    x0 = nc.declare_dram_parameter("x0", [H, W], f32, isOutput=False)
    x1 = nc.declare_dram_parameter("x1", [H, W], i32, isOutput=False)
    y = nc.declare_dram_parameter("y", [H, W], f32, isOutput=True)

    # DRAM viewed with rows grouped by 8: [256, 8, 2048]
    x0g = x0[:].rearrange("(r e) w -> r e w", e=8)
    x1g = x1[:].rearrange("(r e) w -> r e w", e=8)
    yg = y[:].rearrange("(r e) w -> r e w", e=8)

    with tile.TileContext(nc) as tc:
        with ExitStack() as ctx:
            const_pool = ctx.enter_context(tc.tile_pool(name="const", bufs=1))
            io_pool = ctx.enter_context(tc.tile_pool(name="io", bufs=1))
            work = ctx.enter_context(tc.tile_pool(name="work", bufs=1))
            psum = ctx.enter_context(
                tc.tile_pool(name="psum", bufs=4, space="PSUM")
            )

            ident_g = const_pool.tile([P, P], f32)
            make_identity(nc, ident_g[:])
            ident = const_pool.tile([P, P], f32)
            nc.vector.tensor_copy(ident[:], ident_g[:])
            ones = const_pool.tile([P, WD], f32)
            nc.gpsimd.memset(ones[:], 1.0)

            pre_eng = nc.gpsimd if pool_pre else nc.vector
            elem_eng = nc.gpsimd if pool_elem else nc.vector

            def body():
                # ---- tiles ----
                xd = [work.tile([P, WD], f32, tag=f"xd{t}", name=f"xd{t}") for t in range(NT)]
                am = [work.tile([P, WD], f32, tag=f"am{t}", name=f"am{t}") for t in range(NT)]
                xT = [work.tile([P, HD], f32, tag=f"xT{c}", name=f"xT{c}") for c in range(NT)]
                aT = [work.tile([P, HD], f32, tag=f"aT{c}", name=f"aT{c}") for c in range(NT)]
                yh = [None] * NT    # horizontal-pass results
                yv = [None] * NT    # vertical-pass results (transposed)
                ye = [io_pool.tile([P, W], f32, tag=f"ye{t}", name=f"ye{t}") for t in range(NT)]

                def load(t):
                    x0r = io_pool.tile([P, W], f32, tag=f"x0r{t}")
                    x1r = io_pool.tile([P, W], i32, tag=f"x1r{t}")
                    nc.sync.dma_start(
                        out=x0r[:], in_=x0g[t * P:(t + 1) * P, 0, :]
                    )
                    nc.sync.dma_start(
                        out=x1r[:], in_=x1g[t * P:(t + 1) * P, 0, :]
                    )
                    pre_eng.tensor_copy(
                        xd[t][:],
                        x0r[:].rearrange("p (a b) -> p a b", b=8)[:, :, 0],
                    )
                    # (DVE: Pool rejects TensorScalarPtr in codegen)
                    nc.vector.tensor_scalar(
                        out=am[t][:],
                        in0=x1r[:].rearrange("p (a b) -> p a b", b=8)[:, :, 0],
                        scalar1=0, scalar2=None, op0=NE,
                    )

                def seg_pass(xt, at, pf):
                    """One directional pass on tile [P, WD]:
                    returns m = where(q==0, lr+rl, 2*x)."""
                    s_lr = work.tile([P, WD], f32, tag=f"slr{pf}")
                    c_lr = work.tile([P, WD], f32, tag=f"clr{pf}")
                    nb = work.tile([P, WD], f32, tag=f"nb{pf}")
                    s_rl = work.tile([P, WD], f32, tag=f"srl{pf}")  # reversed
                    c_rl = work.tile([P, WD], f32, tag=f"crl{pf}")  # reversed
                    na = work.tile([P, WD], f32, tag=f"na{pf}")     # reversed
                    a_r = _revap(at[:])
                    x_r = _revap(xt[:])
                    ttscan = nc.vector.tensor_tensor_scan
                    ttscan(s_lr[:], at[:], xt[:], 0.0, MUL, ADD)
                    ttscan(c_lr[:], at[:], ones[:], 0.0, MUL, ADD)
                    ttscan(nb[:], at[:], at[:], 1.0, MUL, MUL)
                    ttscan(s_rl[:], a_r, x_r, 0.0, MUL, ADD)
                    ttscan(c_rl[:], a_r, ones[:], 0.0, MUL, ADD)
                    ttscan(na[:], a_r, a_r, 1.0, MUL, MUL)
                    ilr = work.tile([P, WD], f32, tag=f"ilr{pf}")
                    irl = work.tile([P, WD], f32, tag=f"irl{pf}")
                    if fast_recip:
                        nc.vector.reciprocal_approx_fast(ilr[:], c_lr[:])
                        nc.vector.reciprocal_approx_fast(irl[:], c_rl[:])
                    else:
                        scr = work.tile([P, WD], f32, tag=f"scr{pf}")
                        nc.vector.reciprocal_approx_accurate(
                            ilr[:], c_lr[:], scr[:])
                        nc.vector.reciprocal_approx_accurate(
                            irl[:], c_rl[:], scr[:])
                    elem_eng.tensor_tensor(s_lr[:], s_lr[:], ilr[:], MUL)
                    elem_eng.tensor_tensor(s_rl[:], s_rl[:], irl[:], MUL)
                    m = work.tile([P, WD], f32, tag=f"m{pf}")
                    nc.vector.tensor_tensor(m[:], s_lr[:], _revap(s_rl[:]), ADD)
                    # q = nb + reverse(na): nonzero -> no enclosing boundary
                    # (int32: BIR requires an integer predicate dtype)
                    # (DVE: Pool rejects f32->i32 dtype-converting TensorTensor)
                    q = work.tile([P, WD], i32, tag=f"q{pf}")
                    nc.vector.tensor_tensor(q[:], nb[:], _revap(na[:]), ADD)
                    two_x = work.tile([P, WD], f32, tag=f"tx{pf}")
                    nc.scalar.mul(two_x[:], xt[:], 2.0)
                    nc.vector.copy_predicated(m[:], q[:], two_x[:])
                    return m

                def transpose_in(c):
                    """Fill xT[c]/aT[c] (original columns c*P..) from xd/am."""
                    for t in range(NT):
                        for src, dst in ((xd, xT[c]), (am, aT[c])):
                            pb = psum.tile([P, P], f32, tag="ptr")
                            nc.tensor.transpose(
                                pb[:], src[t][:, c * P:(c + 1) * P], ident[:]
                            )
                            nc.scalar.copy(dst[:, t * P:(t + 1) * P], pb[:])

                def combine_store(t, c):
                    """y block (t,c) = yh[t][:,cP:] + yv[c][:,tP:]^T;
                    expand 8x8, store 8 row-replicas."""
                    pb = psum.tile([P, P], f32, tag="ptb")
                    nc.tensor.transpose(
                        pb[:], yv[c][:, t * P:(t + 1) * P], ident[:]
                    )
                    ysum = work.tile([P, P], f32, tag=f"ys{t}{c}")
                    nc.vector.tensor_tensor(
                        ysum[:], yh[t][:, c * P:(c + 1) * P], pb[:], ADD
                    )
                    cw = W // NT
                    nc.scalar.copy(
                        ye[t][:, c * cw:(c + 1) * cw].rearrange(
                            "p (a b) -> p a b", b=8),
                        _bcast8(ysum[:]),
                    )
                    for k in range(8):
                        nc.sync.dma_start(
                            out=yg[t * P:(t + 1) * P, k, c * cw:(c + 1) * cw],
                            in_=ye[t][:, c * cw:(c + 1) * cw],
                        )

                # ---- ordered for earliest store start ----
                load(0)
                load(1)
                yh[0] = seg_pass(xd[0], am[0], "h0")
                transpose_in(0)
                yv[0] = seg_pass(xT[0], aT[0], "v0")
                combine_store(0, 0)           # store stream starts here
                yh[1] = seg_pass(xd[1], am[1], "h1")
                combine_store(1, 0)
                transpose_in(1)
                yv[1] = seg_pass(xT[1], aT[1], "v1")
                combine_store(0, 1)
                combine_store(1, 1)

            if loop_n > 1:
                with tc.For_i(0, loop_n, 1):
                    body()
            else:
                body()

    nc.compile()
    return nc


def _get_nc():
    if "nc" not in _CACHE:
        _CACHE["nc"] = build_program()
    return _CACHE["nc"]


def kernel(x0: np.ndarray, x1: np.ndarray) -> np.ndarray:
    from concourse.bass_utils import run_bass_kernel_spmd

    nc = _get_nc()
    n = x0.shape[0]
    in_maps = [
        {"x0": np.ascontiguousarray(x0[i, 0]),
         "x1": np.ascontiguousarray(x1[i, 0])}
        for i in range(n)
    ]
    res = run_bass_kernel_spmd(nc, in_maps, list(range(N_CORES)))
    out = np.stack([res.results[i]["y"] for i in range(n)])
    return out.reshape(n, 1, H, W).astype(np.float32)


# revision 28
# speedup vs baseline: 1.1899x; 1.0380x over previous
"""Trainium2 Bass kernel for nn_BAFM_BRB_65249143161427 (segment_reduce).

Reference semantics: downsample x0/x1 by 8 (nearest), compute directional
running segment means between zero-boundaries of x1 along rows and columns,
sum the 4 directional terms, upsample by 8.

Sharding: pure data parallel — image n -> core n (N=8, 8 cores).
Each core processes one full 2048x2048 image.

Structure (per core): the 16 MB output store stream dominates (~47 us at
360 GB/s), so the program is ordered to start storing as early as possible:
load -> h-pass(tile0) -> transposes -> v-pass(col0) -> store block (0,0),
with the remaining passes' DVE work hidden under the store stream.
"""
import sys

sys.path.insert(0, "/opt/trn_rl_repo")

import numpy as np

H = W = 2048
S = 8
HD, WD = H // S, W // S      # 256 x 256 downsampled grid
P = 128                      # SBUF partitions
NT = HD // P                 # 2 row-tiles of the downsampled grid
N_CORES = 8

_CACHE = {}


def _revap(ap2d):
    """Reverse the last (free) dim of a 2D access pattern."""
    from concourse.ap import AP

    pairs = [list(p) for p in ap2d.ap]
    step, n = pairs[-1]
    return AP(ap2d.tensor, ap2d.offset + (n - 1) * step, pairs[:-1] + [[-step, n]])


def _bcast8(ap2d):
    """Append a step-0 count-8 inner dim (each element replicated 8x)."""
    from concourse.ap import AP

    pairs = [list(p) for p in ap2d.ap]
    return AP(ap2d.tensor, ap2d.offset, pairs + [[0, 8]])


def build_program(loop_n: int = 1, fast_recip=True, pool_elem=True,
                  pool_pre=True):
    import concourse.bacc as bacc
    import concourse.tile as tile
    from concourse import mybir
    from concourse.masks import make_identity
    from contextlib import ExitStack

    f32 = mybir.dt.float32
    i32 = mybir.dt.int32
    MUL = mybir.AluOpType.mult
    ADD = mybir.AluOpType.add
    NE = mybir.AluOpType.not_equal

    # Bacc (not raw Bass): its compile() splits multi-wait sync commands,
    # which TRN2 engines (1 wait/instruction) require.
    nc = bacc.Bacc("TRN2")
    x0 = nc.declare_dram_parameter("x0", [H, W], f32, isOutput=False)
    x1 = nc.declare_dram_parameter("x1", [H, W], i32, isOutput=False)
    y = nc.declare_dram_parameter("y", [H, W], f32, isOutput=True)

    # DRAM viewed with rows grouped by 8: [256, 8, 2048]
    x0g = x0[:].rearrange("(r e) w -> r e w", e=8)
    x1g = x1[:].rearrange("(r e) w -> r e w", e=8)
    yg = y[:].rearrange("(r e) w -> r e w", e=8)

    with tile.TileContext(nc) as tc:
        with ExitStack() as ctx:
            const_pool = ctx.enter_context(tc.tile_pool(name="const", bufs=1))
            io_pool = ctx.enter_context(tc.tile_pool(name="io", bufs=1))
            work = ctx.enter_context(tc.tile_pool(name="work", bufs=1))
            psum = ctx.enter_context(
                tc.tile_pool(name="psum", bufs=2, space="PSUM")
            )
            psum_x = ctx.enter_context(
                tc.tile_pool(name="psum_x", bufs=2, space="PSUM")
            )

            ident_g = const_pool.tile([P, P], f32)
            make_identity(nc, ident_g[:])
            ident = const_pool.tile([P, P], f32)
            nc.vector.tensor_copy(ident[:], ident_g[:])
            ones = const_pool.tile([P, WD], f32)
            nc.gpsimd.memset(ones[:], 1.0)

            pre_eng = nc.gpsimd if pool_pre else nc.vector
            elem_eng = nc.gpsimd if pool_elem else nc.vector

            def body():
                # ---- tiles ----
                xd = [work.tile([P, WD], f32, tag=f"xd{t}", name=f"xd{t}") for t in range(NT)]
                am = [work.tile([P, WD], f32, tag=f"am{t}", name=f"am{t}") for t in range(NT)]
                # transposed x stays in PSUM (scans read it directly);
                # only the transposed mask is staged to SBUF
                xT = [psum_x.tile([P, HD], f32, tag=f"xT{c}", name=f"xT{c}") for c in range(NT)]
                aT = [work.tile([P, HD], f32, tag=f"aT{c}", name=f"aT{c}") for c in range(NT)]
                yh = [None] * NT    # horizontal-pass results
                yv = [None] * NT    # vertical-pass results (transposed)
                ye = [io_pool.tile([P, W], f32, tag=f"ye{t}", name=f"ye{t}") for t in range(NT)]

                def load(t):
                    x0r = io_pool.tile([P, W], f32, tag=f"x0r{t}")
                    x1r = io_pool.tile([P, W], i32, tag=f"x1r{t}")
                    # mask tensor first: it gates 4 of the 6 scans
                    nc.sync.dma_start(
                        out=x1r[:], in_=x1g[t * P:(t + 1) * P, 0, :]
                    )
                    nc.sync.dma_start(
                        out=x0r[:], in_=x0g[t * P:(t + 1) * P, 0, :]
                    )
                    # (DVE: Pool rejects TensorScalarPtr in codegen)
                    nc.vector.tensor_scalar(
                        out=am[t][:],
                        in0=x1r[:].rearrange("p (a b) -> p a b", b=8)[:, :, 0],
                        scalar1=0, scalar2=None, op0=NE,
                    )
                    pre_eng.tensor_copy(
                        xd[t][:],
                        x0r[:].rearrange("p (a b) -> p a b", b=8)[:, :, 0],
                    )

                def seg_pass(xt, at, pf):
                    """One directional pass on tile [P, WD]:
                    returns m = where(q==0, lr+rl, 2*x)."""
                    s_lr = work.tile([P, WD], f32, tag=f"slr{pf}")
                    c_lr = work.tile([P, WD], f32, tag=f"clr{pf}")
                    nb = work.tile([P, WD], f32, tag=f"nb{pf}")
                    s_rl = work.tile([P, WD], f32, tag=f"srl{pf}")  # reversed
                    c_rl = work.tile([P, WD], f32, tag=f"crl{pf}")  # reversed
                    na = work.tile([P, WD], f32, tag=f"na{pf}")     # reversed
                    a_r = _revap(at[:])
                    x_r = _revap(xt[:])
                    ttscan = nc.vector.tensor_tensor_scan
                    # count scans first so the reciprocals overlap the rest
                    ttscan(c_lr[:], at[:], ones[:], 0.0, MUL, ADD)
                    ttscan(c_rl[:], a_r, ones[:], 0.0, MUL, ADD)
                    ttscan(s_lr[:], at[:], xt[:], 0.0, MUL, ADD)
                    ttscan(s_rl[:], a_r, x_r, 0.0, MUL, ADD)
                    ttscan(nb[:], at[:], at[:], 1.0, MUL, MUL)
                    ttscan(na[:], a_r, a_r, 1.0, MUL, MUL)
                    ilr = work.tile([P, WD], f32, tag=f"ilr{pf}")
                    irl = work.tile([P, WD], f32, tag=f"irl{pf}")
                    if fast_recip:
                        nc.vector.reciprocal_approx_fast(ilr[:], c_lr[:])
                        nc.vector.reciprocal_approx_fast(irl[:], c_rl[:])
                    else:
                        scr = work.tile([P, WD], f32, tag=f"scr{pf}")
                        nc.vector.reciprocal_approx_accurate(
                            ilr[:], c_lr[:], scr[:])
                        nc.vector.reciprocal_approx_accurate(
                            irl[:], c_rl[:], scr[:])
                    elem_eng.tensor_tensor(s_lr[:], s_lr[:], ilr[:], MUL)
                    elem_eng.tensor_tensor(s_rl[:], s_rl[:], irl[:], MUL)
                    m = work.tile([P, WD], f32, tag=f"m{pf}")
                    nc.vector.tensor_tensor(m[:], s_lr[:], _revap(s_rl[:]), ADD)
                    # q = nb + reverse(na): nonzero -> no enclosing boundary
                    # (int32: BIR requires an integer predicate dtype)
                    # (DVE: Pool rejects f32->i32 dtype-converting TensorTensor)
                    q = work.tile([P, WD], i32, tag=f"q{pf}")
                    nc.vector.tensor_tensor(q[:], nb[:], _revap(na[:]), ADD)
                    two_x = work.tile([P, WD], f32, tag=f"tx{pf}")
                    nc.scalar.mul(two_x[:], xt[:], 2.0)
                    nc.vector.copy_predicated(m[:], q[:], two_x[:])
                    return m

                def transpose_in(c):
                    """Fill xT[c] (PSUM) / aT[c] (SBUF) for columns c*P..."""
                    for t in range(NT):
                        # x: transpose straight into the PSUM tile the
                        # vertical scans will read
                        nc.tensor.transpose(
                            xT[c][:, t * P:(t + 1) * P],
                            xd[t][:, c * P:(c + 1) * P], ident[:],
                        )
                        # mask: bounce through PSUM to SBUF (scan data0 and
                        # data1 cannot both live in PSUM)
                        pb = psum.tile([P, P], f32, tag="ptr")
                        nc.tensor.transpose(
                            pb[:], am[t][:, c * P:(c + 1) * P], ident[:]
                        )
                        nc.scalar.copy(aT[c][:, t * P:(t + 1) * P], pb[:])

                def combine_store(t, c):
                    """y block (t,c) = yh[t][:,cP:] + yv[c][:,tP:]^T;
                    expand 8x8, store 8 row-replicas."""
                    pb = psum.tile([P, P], f32, tag="ptb")
                    nc.tensor.transpose(
                        pb[:], yv[c][:, t * P:(t + 1) * P], ident[:]
                    )
                    ysum = work.tile([P, P], f32, tag=f"ys{t}{c}")
                    nc.vector.tensor_tensor(
                        ysum[:], yh[t][:, c * P:(c + 1) * P], pb[:], ADD
                    )
                    cw = W // NT
                    nc.scalar.copy(
                        ye[t][:, c * cw:(c + 1) * cw].rearrange(
                            "p (a b) -> p a b", b=8),
                        _bcast8(ysum[:]),
                    )
                    for k in range(8):
                        nc.sync.dma_start(
                            out=yg[t * P:(t + 1) * P, k, c * cw:(c + 1) * cw],
                            in_=ye[t][:, c * cw:(c + 1) * cw],
                        )

                # ---- ordered for earliest store start ----
                load(0)
                load(1)
                yh[0] = seg_pass(xd[0], am[0], "h0")
                transpose_in(0)
                yv[0] = seg_pass(xT[0], aT[0], "v0")
                combine_store(0, 0)           # store stream starts here
                yh[1] = seg_pass(xd[1], am[1], "h1")
                combine_store(1, 0)
                transpose_in(1)
                yv[1] = seg_pass(xT[1], aT[1], "v1")
                combine_store(0, 1)
                combine_store(1, 1)

            if loop_n > 1:
                with tc.For_i(0, loop_n, 1):
                    body()
            else:
                body()

    nc.compile()
    return nc


def _get_nc():
    if "nc" not in _CACHE:
        _CACHE["nc"] = build_program()
    return _CACHE["nc"]


def kernel(x0: np.ndarray, x1: np.ndarray) -> np.ndarray:
    from concourse.bass_utils import run_bass_kernel_spmd

    nc = _get_nc()
    n = x0.shape[0]
    in_maps = [
        {"x0": np.ascontiguousarray(x0[i, 0]),
         "x1": np.ascontiguousarray(x1[i, 0])}
        for i in range(n)
    ]
    res = run_bass_kernel_spmd(nc, in_maps, list(range(N_CORES)))
    out = np.stack([res.results[i]["y"] for i in range(n)])
    return out.reshape(n, 1, H, W).astype(np.float32)


# revision 44
# speedup vs baseline: 1.2264x; 1.0307x over previous
"""Trainium2 Bass kernel for nn_BAFM_BRB_65249143161427 (segment_reduce).

Reference semantics: downsample x0/x1 by 8 (nearest), compute directional
running segment means between zero-boundaries of x1 along rows and columns,
sum the 4 directional terms, upsample by 8.

Sharding: pure data parallel — image n -> core n (N=8, 8 cores).
Each core processes one full 2048x2048 image.

Structure (per core): the 16 MB output store stream dominates (~47 us at
360 GB/s), so the program is ordered to start storing as early as possible:
load -> h-pass(tile0) -> transposes -> v-pass(col0) -> store block (0,0),
with the remaining passes' DVE work hidden under the store stream.
"""
import sys

sys.path.insert(0, "/opt/trn_rl_repo")

import numpy as np

H = W = 2048
S = 8
HD, WD = H // S, W // S      # 256 x 256 downsampled grid
P = 128                      # SBUF partitions
NT = HD // P                 # 2 row-tiles of the downsampled grid
N_CORES = 8

_CACHE = {}


def _revap(ap2d):
    """Reverse the last (free) dim of a 2D access pattern."""
    from concourse.ap import AP

    pairs = [list(p) for p in ap2d.ap]
    step, n = pairs[-1]
    return AP(ap2d.tensor, ap2d.offset + (n - 1) * step, pairs[:-1] + [[-step, n]])


def _bcast8(ap2d):
    """Append a step-0 count-8 inner dim (each element replicated 8x)."""
    from concourse.ap import AP

    pairs = [list(p) for p in ap2d.ap]
    return AP(ap2d.tensor, ap2d.offset, pairs + [[0, 8]])


def build_program(loop_n: int = 1, fast_recip=True, pool_elem=True,
                  pool_pre=True):
    import concourse.bacc as bacc
    import concourse.tile as tile
    from concourse import mybir
    from concourse.masks import make_identity
    from contextlib import ExitStack

    f32 = mybir.dt.float32
    i32 = mybir.dt.int32
    MUL = mybir.AluOpType.mult
    ADD = mybir.AluOpType.add
    NE = mybir.AluOpType.not_equal

    # Bacc (not raw Bass): its compile() splits multi-wait sync commands,
    # which TRN2 engines (1 wait/instruction) require.
    nc = bacc.Bacc("TRN2")
    x0 = nc.declare_dram_parameter("x0", [H, W], f32, isOutput=False)
    x1 = nc.declare_dram_parameter("x1", [H, W], i32, isOutput=False)
    y = nc.declare_dram_parameter("y", [H, W], f32, isOutput=True)

    # DRAM viewed with rows grouped by 8: [256, 8, 2048]
    x0g = x0[:].rearrange("(r e) w -> r e w", e=8)
    x1g = x1[:].rearrange("(r e) w -> r e w", e=8)
    yg = y[:].rearrange("(r e) w -> r e w", e=8)

    with tile.TileContext(nc) as tc:
        with ExitStack() as ctx:
            const_pool = ctx.enter_context(tc.tile_pool(name="const", bufs=1))
            io_pool = ctx.enter_context(tc.tile_pool(name="io", bufs=1))
            work = ctx.enter_context(tc.tile_pool(name="work", bufs=1))
            psum = ctx.enter_context(
                tc.tile_pool(name="psum", bufs=2, space="PSUM")
            )
            psum_x = ctx.enter_context(
                tc.tile_pool(name="psum_x", bufs=2, space="PSUM")
            )

            ident_g = const_pool.tile([P, P], f32)
            make_identity(nc, ident_g[:])
            ident = const_pool.tile([P, P], f32)
            nc.vector.tensor_copy(ident[:], ident_g[:])
            ones = const_pool.tile([P, WD], f32)
            nc.gpsimd.memset(ones[:], 1.0)

            pre_eng = nc.gpsimd if pool_pre else nc.vector
            elem_eng = nc.gpsimd if pool_elem else nc.vector

            def body():
                # ---- tiles ----
                xd = [work.tile([P, WD], f32, tag=f"xd{t}", name=f"xd{t}") for t in range(NT)]
                am = [work.tile([P, WD], f32, tag=f"am{t}", name=f"am{t}") for t in range(NT)]
                # transposed x stays in PSUM (scans read it directly);
                # only the transposed mask is staged to SBUF
                xT = [psum_x.tile([P, HD], f32, tag=f"xT{c}", name=f"xT{c}") for c in range(NT)]
                aT = [work.tile([P, HD], f32, tag=f"aT{c}", name=f"aT{c}") for c in range(NT)]
                yh = [None] * NT    # horizontal-pass results
                yv = [None] * NT    # vertical-pass results (transposed)
                ye = [io_pool.tile([P, W], f32, tag=f"ye{t}", name=f"ye{t}") for t in range(NT)]

                def load_x1(t):
                    x1r = io_pool.tile([P, W], i32, tag=f"x1r{t}")
                    nc.sync.dma_start(
                        out=x1r[:], in_=x1g[t * P:(t + 1) * P, 0, :]
                    )
                    # (DVE: Pool rejects TensorScalarPtr in codegen)
                    nc.vector.tensor_scalar(
                        out=am[t][:],
                        in0=x1r[:].rearrange("p (a b) -> p a b", b=8)[:, :, 0],
                        scalar1=0, scalar2=None, op0=NE,
                    )

                x0r = [None] * NT

                def load_x0_half(t, h):
                    # half-column DMA + half-granular downsample copy: the
                    # left halves of both xd tiles (which gate the first
                    # vertical transpose) load before either right half
                    if x0r[t] is None:
                        x0r[t] = io_pool.tile(
                            [P, W], f32, tag=f"x0r{t}", name=f"x0r{t}")
                    hw_ = W // 2
                    hd_ = WD // 2
                    nc.sync.dma_start(
                        out=x0r[t][:, h * hw_:(h + 1) * hw_],
                        in_=x0g[t * P:(t + 1) * P, 0, h * hw_:(h + 1) * hw_],
                    )
                    pre_eng.tensor_copy(
                        xd[t][:, h * hd_:(h + 1) * hd_],
                        x0r[t][:, h * hw_:(h + 1) * hw_].rearrange(
                            "p (a b) -> p a b", b=8)[:, :, 0],
                    )

                def seg_counts(at, pf):
                    """Mask-only stage: count reciprocals + invalid mask q."""
                    c_lr = work.tile([P, WD], f32, tag=f"clr{pf}")
                    nb = work.tile([P, WD], f32, tag=f"nb{pf}")
                    c_rl = work.tile([P, WD], f32, tag=f"crl{pf}")  # reversed
                    na = work.tile([P, WD], f32, tag=f"na{pf}")     # reversed
                    a_r = _revap(at[:])
                    ttscan = nc.vector.tensor_tensor_scan
                    ttscan(c_lr[:], at[:], ones[:], 0.0, MUL, ADD)
                    ttscan(c_rl[:], a_r, ones[:], 0.0, MUL, ADD)
                    ttscan(nb[:], at[:], at[:], 1.0, MUL, MUL)
                    ttscan(na[:], a_r, a_r, 1.0, MUL, MUL)
                    ilr = work.tile([P, WD], f32, tag=f"ilr{pf}")
                    irl = work.tile([P, WD], f32, tag=f"irl{pf}")
                    if fast_recip:
                        nc.vector.reciprocal_approx_fast(ilr[:], c_lr[:])
                        nc.vector.reciprocal_approx_fast(irl[:], c_rl[:])
                    else:
                        scr = work.tile([P, WD], f32, tag=f"scr{pf}")
                        nc.vector.reciprocal_approx_accurate(
                            ilr[:], c_lr[:], scr[:])
                        nc.vector.reciprocal_approx_accurate(
                            irl[:], c_rl[:], scr[:])
                    # q = nb + reverse(na): nonzero -> no enclosing boundary
                    # (int32: BIR requires an integer predicate dtype)
                    # (DVE: Pool rejects f32->i32 dtype-converting TensorTensor)
                    q = work.tile([P, WD], i32, tag=f"q{pf}")
                    nc.vector.tensor_tensor(q[:], nb[:], _revap(na[:]), ADD)
                    return ilr, irl, q

                def seg_sums(xt, at, cnts, pf):
                    """Value stage: segment sums -> means -> m with fallback."""
                    ilr, irl, q = cnts
                    s_lr = work.tile([P, WD], f32, tag=f"slr{pf}")
                    s_rl = work.tile([P, WD], f32, tag=f"srl{pf}")  # reversed
                    a_r = _revap(at[:])
                    x_r = _revap(xt[:])
                    ttscan = nc.vector.tensor_tensor_scan
                    ttscan(s_lr[:], at[:], xt[:], 0.0, MUL, ADD)
                    ttscan(s_rl[:], a_r, x_r, 0.0, MUL, ADD)
                    elem_eng.tensor_tensor(s_lr[:], s_lr[:], ilr[:], MUL)
                    elem_eng.tensor_tensor(s_rl[:], s_rl[:], irl[:], MUL)
                    m = work.tile([P, WD], f32, tag=f"m{pf}")
                    nc.vector.tensor_tensor(m[:], s_lr[:], _revap(s_rl[:]), ADD)
                    two_x = work.tile([P, WD], f32, tag=f"tx{pf}")
                    nc.scalar.mul(two_x[:], xt[:], 2.0)
                    nc.vector.copy_predicated(m[:], q[:], two_x[:])
                    return m

                def seg_sums_split(xt, at, cnts, pf, tail_eng=None):
                    """Critical-path variant: tail split into column halves —
                    half 0 (needed by the first store block) first; second
                    half's arithmetic goes to Pool to keep DVE clear.
                    Returns (m, finish_fn)."""
                    ilr, irl, q = cnts
                    s_lr = work.tile([P, WD], f32, tag=f"slr{pf}")
                    s_rl = work.tile([P, WD], f32, tag=f"srl{pf}")  # reversed
                    ttscan = nc.vector.tensor_tensor_scan
                    ttscan(s_lr[:], at[:], xt[:], 0.0, MUL, ADD)
                    ttscan(s_rl[:], _revap(at[:]), _revap(xt[:]), 0.0, MUL, ADD)
                    m = work.tile([P, WD], f32, tag=f"m{pf}")
                    two_x = work.tile([P, WD], f32, tag=f"tx{pf}")
                    nc.scalar.mul(two_x[:], xt[:], 2.0)
                    hd_ = WD // 2

                    def half(h, eng):
                        lo, hi = h * hd_, (h + 1) * hd_
                        rlo, rhi = WD - hi, WD - lo  # mirrored slice (rev space)
                        eng.tensor_tensor(
                            s_lr[:, lo:hi], s_lr[:, lo:hi], ilr[:, lo:hi], MUL)
                        eng.tensor_tensor(
                            s_rl[:, rlo:rhi], s_rl[:, rlo:rhi],
                            irl[:, rlo:rhi], MUL)
                        eng.tensor_tensor(
                            m[:, lo:hi], s_lr[:, lo:hi],
                            _revap(s_rl[:, rlo:rhi]), ADD)
                        nc.vector.copy_predicated(
                            m[:, lo:hi], q[:, lo:hi], two_x[:, lo:hi])

                    half(0, tail_eng or nc.vector)
                    return m, (lambda: half(1, nc.vector))

                def transpose_a(c):
                    """Transposed mask -> aT[c] (SBUF, bounced via PSUM:
                    scan data0 and data1 cannot both live in PSUM)."""
                    for t in range(NT):
                        pb = psum.tile([P, P], f32, tag="ptr")
                        nc.tensor.transpose(
                            pb[:], am[t][:, c * P:(c + 1) * P], ident[:]
                        )
                        nc.scalar.copy(aT[c][:, t * P:(t + 1) * P], pb[:])

                def transpose_x(c):
                    """Transpose x straight into the PSUM tile the vertical
                    sum-scans read."""
                    for t in range(NT):
                        nc.tensor.transpose(
                            xT[c][:, t * P:(t + 1) * P],
                            xd[t][:, c * P:(c + 1) * P], ident[:],
                        )

                def combine_store(t, c, first=False):
                    """y block (t,c) = yh[t][:,cP:] + yv[c][:,tP:]^T;
                    expand 8x8, store 8 row-replicas. first=True keeps the
                    expansion on DVE (skips the ACT hop on the path that
                    opens the store stream)."""
                    pb = psum.tile([P, P], f32, tag="ptb")
                    nc.tensor.transpose(
                        pb[:], yv[c][:, t * P:(t + 1) * P], ident[:]
                    )
                    ysum = work.tile([P, P], f32, tag=f"ys{t}{c}")
                    nc.vector.tensor_tensor(
                        ysum[:], yh[t][:, c * P:(c + 1) * P], pb[:], ADD
                    )
                    cw = W // NT
                    exp_eng = nc.vector if first else nc.scalar
                    if first:
                        exp_eng.tensor_copy(
                            ye[t][:, c * cw:(c + 1) * cw].rearrange(
                                "p (a b) -> p a b", b=8),
                            _bcast8(ysum[:]),
                        )
                    else:
                        exp_eng.copy(
                            ye[t][:, c * cw:(c + 1) * cw].rearrange(
                                "p (a b) -> p a b", b=8),
                            _bcast8(ysum[:]),
                        )
                    for k in range(8):
                        nc.sync.dma_start(
                            out=yg[t * P:(t + 1) * P, k, c * cw:(c + 1) * cw],
                            in_=ye[t][:, c * cw:(c + 1) * cw],
                        )

                # ---- ordered for earliest store start ----
                # masks first (x1 loads lead), count stages run during x0
                # loads, then sum stages feed the store stream
                load_x1(0)
                load_x1(1)
                load_x0_half(0, 0)
                load_x0_half(0, 1)
                load_x0_half(1, 0)
                load_x0_half(1, 1)
                ch0 = seg_counts(am[0], "h0")
                transpose_a(0)
                cv0 = seg_counts(aT[0], "v0")
                # the chain that opens the store stream gets top scheduling
                # priority so ready-but-noncritical work can't delay it
                with tc.high_priority():
                    transpose_x(0)
                    yv[0], fin_v0 = seg_sums_split(xT[0], aT[0], cv0, "v0")
                    yh[0], fin_h0 = seg_sums_split(xd[0], am[0], ch0, "h0")
                    combine_store(0, 0, first=True)  # store stream opens
                fin_v0()
                fin_h0()
                ch1 = seg_counts(am[1], "h1")
                yh[1] = seg_sums(xd[1], am[1], ch1, "h1")
                combine_store(1, 0)
                transpose_a(1)
                cv1 = seg_counts(aT[1], "v1")
                transpose_x(1)
                yv[1] = seg_sums(xT[1], aT[1], cv1, "v1")
                combine_store(0, 1)
                combine_store(1, 1)

            if loop_n > 1:
                with tc.For_i(0, loop_n, 1):
                    body()
            else:
                body()

    nc.compile()
    return nc


def _get_nc():
    if "nc" not in _CACHE:
        _CACHE["nc"] = build_program()
    return _CACHE["nc"]


def kernel(x0: np.ndarray, x1: np.ndarray) -> np.ndarray:
    from concourse.bass_utils import run_bass_kernel_spmd

    nc = _get_nc()
    n = x0.shape[0]
    in_maps = [
        {"x0": np.ascontiguousarray(x0[i, 0]),
         "x1": np.ascontiguousarray(x1[i, 0])}
        for i in range(n)
    ]
    res = run_bass_kernel_spmd(nc, in_maps, list(range(N_CORES)))
    out = np.stack([res.results[i]["y"] for i in range(n)])
    return out.reshape(n, 1, H, W).astype(np.float32)


# revision 48
# speedup vs baseline: 1.3217x; 1.0777x over previous
"""Trainium2 Bass kernel for nn_BAFM_BRB_65249143161427 (segment_reduce).

Reference semantics: downsample x0/x1 by 8 (nearest), compute directional
running segment means between zero-boundaries of x1 along rows and columns,
sum the 4 directional terms, upsample by 8.

Sharding: pure data parallel — image n -> core n (N=8, 8 cores).
Each core processes one full 2048x2048 image.

Structure (per core): the 16 MB output store stream dominates (~47 us at
360 GB/s), so the program is ordered to start storing as early as possible:
load -> h-pass(tile0) -> transposes -> v-pass(col0) -> store block (0,0),
with the remaining passes' DVE work hidden under the store stream.
"""
import sys

sys.path.insert(0, "/opt/trn_rl_repo")

import numpy as np

H = W = 2048
S = 8
HD, WD = H // S, W // S      # 256 x 256 downsampled grid
P = 128                      # SBUF partitions
NT = HD // P                 # 2 row-tiles of the downsampled grid
N_CORES = 8

_CACHE = {}


def _revap(ap2d):
    """Reverse the last (free) dim of a 2D access pattern."""
    from concourse.ap import AP

    pairs = [list(p) for p in ap2d.ap]
    step, n = pairs[-1]
    return AP(ap2d.tensor, ap2d.offset + (n - 1) * step, pairs[:-1] + [[-step, n]])


def _bcast8(ap2d):
    """Append a step-0 count-8 inner dim (each element replicated 8x)."""
    from concourse.ap import AP

    pairs = [list(p) for p in ap2d.ap]
    return AP(ap2d.tensor, ap2d.offset, pairs + [[0, 8]])


def build_program(loop_n: int = 1, fast_recip=True, pool_elem=True,
                  pool_pre=True):
    import concourse.bacc as bacc
    import concourse.tile as tile
    from concourse import mybir
    from concourse.masks import make_identity
    from contextlib import ExitStack

    f32 = mybir.dt.float32
    i32 = mybir.dt.int32
    MUL = mybir.AluOpType.mult
    ADD = mybir.AluOpType.add
    NE = mybir.AluOpType.not_equal

    # Bacc (not raw Bass): its compile() splits multi-wait sync commands,
    # which TRN2 engines (1 wait/instruction) require.
    nc = bacc.Bacc("TRN2")
    x0 = nc.declare_dram_parameter("x0", [H, W], f32, isOutput=False)
    x1 = nc.declare_dram_parameter("x1", [H, W], i32, isOutput=False)
    y = nc.declare_dram_parameter("y", [H, W], f32, isOutput=True)

    # DRAM viewed with rows grouped by 8: [256, 8, 2048]
    x0g = x0[:].rearrange("(r e) w -> r e w", e=8)
    x1g = x1[:].rearrange("(r e) w -> r e w", e=8)
    yg = y[:].rearrange("(r e) w -> r e w", e=8)

    with tile.TileContext(nc) as tc:
        with ExitStack() as ctx:
            const_pool = ctx.enter_context(tc.tile_pool(name="const", bufs=1))
            io_pool = ctx.enter_context(tc.tile_pool(name="io", bufs=1))
            work = ctx.enter_context(tc.tile_pool(name="work", bufs=1))
            psum = ctx.enter_context(
                tc.tile_pool(name="psum", bufs=2, space="PSUM")
            )
            psum_x = ctx.enter_context(
                tc.tile_pool(name="psum_x", bufs=2, space="PSUM")
            )

            ident_g = const_pool.tile([P, P], f32)
            make_identity(nc, ident_g[:])
            ident = const_pool.tile([P, P], f32)
            nc.vector.tensor_copy(ident[:], ident_g[:])
            ones = const_pool.tile([P, WD], f32)
            nc.gpsimd.memset(ones[:], 1.0)

            pre_eng = nc.gpsimd if pool_pre else nc.vector
            elem_eng = nc.gpsimd if pool_elem else nc.vector

            def body():
                # ---- tiles ----
                am = [work.tile([P, WD], f32, tag=f"am{t}", name=f"am{t}") for t in range(NT)]
                # transposed x stays in PSUM (scans read it directly);
                # only the transposed mask is staged to SBUF
                xT = [psum_x.tile([P, HD], f32, tag=f"xT{c}", name=f"xT{c}") for c in range(NT)]
                aT = [work.tile([P, HD], f32, tag=f"aT{c}", name=f"aT{c}") for c in range(NT)]
                yh = [None] * NT    # horizontal-pass results
                yv = [None] * NT    # vertical-pass results (transposed)
                ye = [io_pool.tile([P, W], f32, tag=f"ye{t}", name=f"ye{t}") for t in range(NT)]

                def load_x1(t):
                    x1r = io_pool.tile([P, W], i32, tag=f"x1r{t}")
                    nc.sync.dma_start(
                        out=x1r[:], in_=x1g[t * P:(t + 1) * P, 0, :]
                    )
                    # (DVE: Pool rejects TensorScalarPtr in codegen)
                    nc.vector.tensor_scalar(
                        out=am[t][:],
                        in0=x1r[:].rearrange("p (a b) -> p a b", b=8)[:, :, 0],
                        scalar1=0, scalar2=None, op0=NE,
                    )

                x0r = [None] * NT

                def load_x0_half(t, h):
                    # half-column DMA; consumers read the row tile directly
                    # through stride-8 views (no downsample copy, no extra
                    # DMA-sem hop on the critical path)
                    if x0r[t] is None:
                        x0r[t] = io_pool.tile(
                            [P, W], f32, tag=f"x0r{t}", name=f"x0r{t}")
                    hw_ = W // 2
                    nc.sync.dma_start(
                        out=x0r[t][:, h * hw_:(h + 1) * hw_],
                        in_=x0g[t * P:(t + 1) * P, 0, h * hw_:(h + 1) * hw_],
                    )

                def xdv(t, lo=0, hi=WD):
                    """Stride-8 view of x0r[t] covering downsampled cols
                    [lo, hi)."""
                    return x0r[t][:, lo * 8:hi * 8].rearrange(
                        "p (a b) -> p a b", b=8)[:, :, 0]

                def seg_counts(at, pf):
                    """Mask-only stage: count reciprocals + invalid mask q."""
                    c_lr = work.tile([P, WD], f32, tag=f"clr{pf}")
                    nb = work.tile([P, WD], f32, tag=f"nb{pf}")
                    c_rl = work.tile([P, WD], f32, tag=f"crl{pf}")  # reversed
                    na = work.tile([P, WD], f32, tag=f"na{pf}")     # reversed
                    a_r = _revap(at[:])
                    ttscan = nc.vector.tensor_tensor_scan
                    ttscan(c_lr[:], at[:], ones[:], 0.0, MUL, ADD)
                    ttscan(c_rl[:], a_r, ones[:], 0.0, MUL, ADD)
                    ttscan(nb[:], at[:], at[:], 1.0, MUL, MUL)
                    ttscan(na[:], a_r, a_r, 1.0, MUL, MUL)
                    ilr = work.tile([P, WD], f32, tag=f"ilr{pf}")
                    irl = work.tile([P, WD], f32, tag=f"irl{pf}")
                    if fast_recip:
                        nc.vector.reciprocal_approx_fast(ilr[:], c_lr[:])
                        nc.vector.reciprocal_approx_fast(irl[:], c_rl[:])
                    else:
                        scr = work.tile([P, WD], f32, tag=f"scr{pf}")
                        nc.vector.reciprocal_approx_accurate(
                            ilr[:], c_lr[:], scr[:])
                        nc.vector.reciprocal_approx_accurate(
                            irl[:], c_rl[:], scr[:])
                    # q = nb + reverse(na): nonzero -> no enclosing boundary
                    # (int32: BIR requires an integer predicate dtype)
                    # (DVE: Pool rejects f32->i32 dtype-converting TensorTensor)
                    q = work.tile([P, WD], i32, tag=f"q{pf}")
                    nc.vector.tensor_tensor(q[:], nb[:], _revap(na[:]), ADD)
                    return ilr, irl, q

                def seg_sums(xt, at, cnts, pf):
                    """Value stage: segment sums -> means -> m with fallback."""
                    ilr, irl, q = cnts
                    s_lr = work.tile([P, WD], f32, tag=f"slr{pf}")
                    s_rl = work.tile([P, WD], f32, tag=f"srl{pf}")  # reversed
                    a_r = _revap(at[:])
                    x_r = _revap(xt[:])
                    ttscan = nc.vector.tensor_tensor_scan
                    ttscan(s_lr[:], at[:], xt[:], 0.0, MUL, ADD)
                    ttscan(s_rl[:], a_r, x_r, 0.0, MUL, ADD)
                    elem_eng.tensor_tensor(s_lr[:], s_lr[:], ilr[:], MUL)
                    elem_eng.tensor_tensor(s_rl[:], s_rl[:], irl[:], MUL)
                    m = work.tile([P, WD], f32, tag=f"m{pf}")
                    nc.vector.tensor_tensor(m[:], s_lr[:], _revap(s_rl[:]), ADD)
                    two_x = work.tile([P, WD], f32, tag=f"tx{pf}")
                    nc.scalar.mul(two_x[:], xt[:], 2.0)
                    nc.vector.copy_predicated(m[:], q[:], two_x[:])
                    return m

                def seg_sums_split(xt, at, cnts, pf, tail_eng=None):
                    """Critical-path variant: tail split into column halves —
                    half 0 (needed by the first store block) first; second
                    half's arithmetic goes to Pool to keep DVE clear.
                    Returns (m, finish_fn)."""
                    ilr, irl, q = cnts
                    s_lr = work.tile([P, WD], f32, tag=f"slr{pf}")
                    s_rl = work.tile([P, WD], f32, tag=f"srl{pf}")  # reversed
                    ttscan = nc.vector.tensor_tensor_scan
                    ttscan(s_lr[:], at[:], xt[:], 0.0, MUL, ADD)
                    ttscan(s_rl[:], _revap(at[:]), _revap(xt[:]), 0.0, MUL, ADD)
                    m = work.tile([P, WD], f32, tag=f"m{pf}")
                    two_x = work.tile([P, WD], f32, tag=f"tx{pf}")
                    nc.scalar.mul(two_x[:], xt[:], 2.0)
                    hd_ = WD // 2

                    def half(h, eng):
                        lo, hi = h * hd_, (h + 1) * hd_
                        rlo, rhi = WD - hi, WD - lo  # mirrored slice (rev space)
                        eng.tensor_tensor(
                            s_lr[:, lo:hi], s_lr[:, lo:hi], ilr[:, lo:hi], MUL)
                        eng.tensor_tensor(
                            s_rl[:, rlo:rhi], s_rl[:, rlo:rhi],
                            irl[:, rlo:rhi], MUL)
                        eng.tensor_tensor(
                            m[:, lo:hi], s_lr[:, lo:hi],
                            _revap(s_rl[:, rlo:rhi]), ADD)
                        nc.vector.copy_predicated(
                            m[:, lo:hi], q[:, lo:hi], two_x[:, lo:hi])

                    half(0, tail_eng or nc.vector)
                    return m, (lambda: half(1, nc.vector))

                def transpose_a(c):
                    """Transposed mask -> aT[c] (SBUF, bounced via PSUM:
                    scan data0 and data1 cannot both live in PSUM)."""
                    for t in range(NT):
                        pb = psum.tile([P, P], f32, tag="ptr")
                        nc.tensor.transpose(
                            pb[:], am[t][:, c * P:(c + 1) * P], ident[:]
                        )
                        nc.scalar.copy(aT[c][:, t * P:(t + 1) * P], pb[:])

                def transpose_x(c):
                    """Transpose x straight into the PSUM tile the vertical
                    sum-scans read."""
                    for t in range(NT):
                        nc.tensor.transpose(
                            xT[c][:, t * P:(t + 1) * P],
                            xdv(t, c * P, (c + 1) * P), ident[:],
                        )

                def combine_store(t, c, first=False):
                    """y block (t,c) = yh[t][:,cP:] + yv[c][:,tP:]^T;
                    expand 8x8, store 8 row-replicas. first=True keeps the
                    expansion on DVE (skips the ACT hop on the path that
                    opens the store stream)."""
                    pb = psum.tile([P, P], f32, tag="ptb")
                    nc.tensor.transpose(
                        pb[:], yv[c][:, t * P:(t + 1) * P], ident[:]
                    )
                    cw = W // NT
                    ye_view = ye[t][:, c * cw:(c + 1) * cw].rearrange(
                        "p (a b) -> p a b", b=8)
                    if first:
                        ysum = work.tile([P, P], f32, tag=f"ys{t}{c}")
                        nc.vector.tensor_tensor(
                            ysum[:], yh[t][:, c * P:(c + 1) * P], pb[:], ADD
                        )
                        nc.vector.tensor_copy(ye_view, _bcast8(ysum[:]))
                    else:
                        ysum = work.tile([P, P], f32, tag=f"ys{t}{c}")
                        nc.vector.tensor_tensor(
                            ysum[:], yh[t][:, c * P:(c + 1) * P], pb[:], ADD
                        )
                        nc.scalar.copy(ye_view, _bcast8(ysum[:]))
                    for k in range(8):
                        nc.sync.dma_start(
                            out=yg[t * P:(t + 1) * P, k, c * cw:(c + 1) * cw],
                            in_=ye[t][:, c * cw:(c + 1) * cw],
                        )

                # ---- ordered for earliest store start ----
                # masks first (x1 loads lead), count stages run during x0
                # loads, then sum stages feed the store stream
                load_x1(0)
                load_x1(1)
                load_x0_half(0, 0)
                load_x0_half(1, 0)
                load_x0_half(0, 1)
                load_x0_half(1, 1)
                ch0 = seg_counts(am[0], "h0")
                transpose_a(0)
                cv0 = seg_counts(aT[0], "v0")
                # the chain that opens the store stream gets top scheduling
                # priority so ready-but-noncritical work can't delay it
                with tc.high_priority():
                    transpose_x(0)
                    yv[0], fin_v0 = seg_sums_split(xT[0], aT[0], cv0, "v0")
                    yh[0], fin_h0 = seg_sums_split(xdv(0), am[0], ch0, "h0")
                    combine_store(0, 0, first=True)  # store stream opens
                fin_v0()
                fin_h0()
                ch1 = seg_counts(am[1], "h1")
                yh[1] = seg_sums(xdv(1), am[1], ch1, "h1")
                combine_store(1, 0)
                transpose_a(1)
                cv1 = seg_counts(aT[1], "v1")
                transpose_x(1)
                yv[1] = seg_sums(xT[1], aT[1], cv1, "v1")
                combine_store(0, 1)
                combine_store(1, 1)

            if loop_n > 1:
                with tc.For_i(0, loop_n, 1):
                    body()
            else:
                body()

    nc.compile()
    return nc


def _get_nc():
    if "nc" not in _CACHE:
        _CACHE["nc"] = build_program()
    return _CACHE["nc"]


def kernel(x0: np.ndarray, x1: np.ndarray) -> np.ndarray:
    from concourse.bass_utils import run_bass_kernel_spmd

    nc = _get_nc()
    n = x0.shape[0]
    in_maps = [
        {"x0": np.ascontiguousarray(x0[i, 0]),
         "x1": np.ascontiguousarray(x1[i, 0])}
        for i in range(n)
    ]
    res = run_bass_kernel_spmd(nc, in_maps, list(range(N_CORES)))
    out = np.stack([res.results[i]["y"] for i in range(n)])
    return out.reshape(n, 1, H, W).astype(np.float32)


# revision 49
# speedup vs baseline: 1.3534x; 1.0240x over previous
"""Trainium2 Bass kernel for nn_BAFM_BRB_65249143161427 (segment_reduce).

Reference semantics: downsample x0/x1 by 8 (nearest), compute directional
running segment means between zero-boundaries of x1 along rows and columns,
sum the 4 directional terms, upsample by 8.

Sharding: pure data parallel — image n -> core n (N=8, 8 cores).
Each core processes one full 2048x2048 image.

Structure (per core): the 16 MB output store stream dominates (~47 us at
360 GB/s), so the program is ordered to start storing as early as possible:
load -> h-pass(tile0) -> transposes -> v-pass(col0) -> store block (0,0),
with the remaining passes' DVE work hidden under the store stream.
"""
import sys

sys.path.insert(0, "/opt/trn_rl_repo")

import numpy as np

H = W = 2048
S = 8
HD, WD = H // S, W // S      # 256 x 256 downsampled grid
P = 128                      # SBUF partitions
NT = HD // P                 # 2 row-tiles of the downsampled grid
N_CORES = 8

_CACHE = {}


def _revap(ap2d):
    """Reverse the last (free) dim of a 2D access pattern."""
    from concourse.ap import AP

    pairs = [list(p) for p in ap2d.ap]
    step, n = pairs[-1]
    return AP(ap2d.tensor, ap2d.offset + (n - 1) * step, pairs[:-1] + [[-step, n]])


def _bcast8(ap2d):
    """Append a step-0 count-8 inner dim (each element replicated 8x)."""
    from concourse.ap import AP

    pairs = [list(p) for p in ap2d.ap]
    return AP(ap2d.tensor, ap2d.offset, pairs + [[0, 8]])


def build_program(loop_n: int = 1, fast_recip=False, pool_elem=True,
                  pool_pre=True):
    import concourse.bacc as bacc
    import concourse.tile as tile
    from concourse import mybir
    from concourse.masks import make_identity
    from contextlib import ExitStack

    f32 = mybir.dt.float32
    i32 = mybir.dt.int32
    MUL = mybir.AluOpType.mult
    ADD = mybir.AluOpType.add
    NE = mybir.AluOpType.not_equal

    # Bacc (not raw Bass): its compile() splits multi-wait sync commands,
    # which TRN2 engines (1 wait/instruction) require.
    nc = bacc.Bacc("TRN2")
    x0 = nc.declare_dram_parameter("x0", [H, W], f32, isOutput=False)
    x1 = nc.declare_dram_parameter("x1", [H, W], i32, isOutput=False)
    y = nc.declare_dram_parameter("y", [H, W], f32, isOutput=True)

    # DRAM viewed with rows grouped by 8: [256, 8, 2048]
    x0g = x0[:].rearrange("(r e) w -> r e w", e=8)
    x1g = x1[:].rearrange("(r e) w -> r e w", e=8)
    yg = y[:].rearrange("(r e) w -> r e w", e=8)

    with tile.TileContext(nc) as tc:
        with ExitStack() as ctx:
            const_pool = ctx.enter_context(tc.tile_pool(name="const", bufs=1))
            io_pool = ctx.enter_context(tc.tile_pool(name="io", bufs=1))
            work = ctx.enter_context(tc.tile_pool(name="work", bufs=1))
            psum = ctx.enter_context(
                tc.tile_pool(name="psum", bufs=2, space="PSUM")
            )
            psum_x = ctx.enter_context(
                tc.tile_pool(name="psum_x", bufs=2, space="PSUM")
            )

            ident_g = const_pool.tile([P, P], f32)
            make_identity(nc, ident_g[:])
            ident = const_pool.tile([P, P], f32)
            nc.vector.tensor_copy(ident[:], ident_g[:])
            ones = const_pool.tile([P, WD], f32)
            nc.gpsimd.memset(ones[:], 1.0)

            pre_eng = nc.gpsimd if pool_pre else nc.vector
            elem_eng = nc.gpsimd if pool_elem else nc.vector

            def body():
                # ---- tiles ----
                am = [work.tile([P, WD], f32, tag=f"am{t}", name=f"am{t}") for t in range(NT)]
                # transposed x stays in PSUM (scans read it directly);
                # only the transposed mask is staged to SBUF
                xT = [psum_x.tile([P, HD], f32, tag=f"xT{c}", name=f"xT{c}") for c in range(NT)]
                aT = [work.tile([P, HD], f32, tag=f"aT{c}", name=f"aT{c}") for c in range(NT)]
                yh = [None] * NT    # horizontal-pass results
                yv = [None] * NT    # vertical-pass results (transposed)
                ye = [io_pool.tile([P, W], f32, tag=f"ye{t}", name=f"ye{t}") for t in range(NT)]

                def load_x1(t):
                    x1r = io_pool.tile([P, W], i32, tag=f"x1r{t}")
                    nc.sync.dma_start(
                        out=x1r[:], in_=x1g[t * P:(t + 1) * P, 0, :]
                    )
                    # (DVE: Pool rejects TensorScalarPtr in codegen)
                    nc.vector.tensor_scalar(
                        out=am[t][:],
                        in0=x1r[:].rearrange("p (a b) -> p a b", b=8)[:, :, 0],
                        scalar1=0, scalar2=None, op0=NE,
                    )

                x0r = [None] * NT

                def load_x0_half(t, h):
                    # half-column DMA; consumers read the row tile directly
                    # through stride-8 views (no downsample copy, no extra
                    # DMA-sem hop on the critical path)
                    if x0r[t] is None:
                        x0r[t] = io_pool.tile(
                            [P, W], f32, tag=f"x0r{t}", name=f"x0r{t}")
                    hw_ = W // 2
                    nc.sync.dma_start(
                        out=x0r[t][:, h * hw_:(h + 1) * hw_],
                        in_=x0g[t * P:(t + 1) * P, 0, h * hw_:(h + 1) * hw_],
                    )

                def xdv(t, lo=0, hi=WD):
                    """Stride-8 view of x0r[t] covering downsampled cols
                    [lo, hi)."""
                    return x0r[t][:, lo * 8:hi * 8].rearrange(
                        "p (a b) -> p a b", b=8)[:, :, 0]

                def seg_counts(at, pf):
                    """Mask-only stage: count reciprocals + invalid mask q."""
                    c_lr = work.tile([P, WD], f32, tag=f"clr{pf}")
                    nb = work.tile([P, WD], f32, tag=f"nb{pf}")
                    c_rl = work.tile([P, WD], f32, tag=f"crl{pf}")  # reversed
                    na = work.tile([P, WD], f32, tag=f"na{pf}")     # reversed
                    a_r = _revap(at[:])
                    ttscan = nc.vector.tensor_tensor_scan
                    ttscan(c_lr[:], at[:], ones[:], 0.0, MUL, ADD)
                    ttscan(c_rl[:], a_r, ones[:], 0.0, MUL, ADD)
                    ttscan(nb[:], at[:], at[:], 1.0, MUL, MUL)
                    ttscan(na[:], a_r, a_r, 1.0, MUL, MUL)
                    ilr = work.tile([P, WD], f32, tag=f"ilr{pf}")
                    irl = work.tile([P, WD], f32, tag=f"irl{pf}")
                    if fast_recip:
                        nc.vector.reciprocal_approx_fast(ilr[:], c_lr[:])
                        nc.vector.reciprocal_approx_fast(irl[:], c_rl[:])
                    else:
                        scr = work.tile([P, WD], f32, tag=f"scr{pf}")
                        nc.vector.reciprocal_approx_accurate(
                            ilr[:], c_lr[:], scr[:])
                        nc.vector.reciprocal_approx_accurate(
                            irl[:], c_rl[:], scr[:])
                    # q = nb + reverse(na): nonzero -> no enclosing boundary
                    # (int32: BIR requires an integer predicate dtype)
                    # (DVE: Pool rejects f32->i32 dtype-converting TensorTensor)
                    q = work.tile([P, WD], i32, tag=f"q{pf}")
                    nc.vector.tensor_tensor(q[:], nb[:], _revap(na[:]), ADD)
                    return ilr, irl, q

                def seg_sums(xt, at, cnts, pf):
                    """Value stage: segment sums -> means -> m with fallback."""
                    ilr, irl, q = cnts
                    s_lr = work.tile([P, WD], f32, tag=f"slr{pf}")
                    s_rl = work.tile([P, WD], f32, tag=f"srl{pf}")  # reversed
                    a_r = _revap(at[:])
                    x_r = _revap(xt[:])
                    ttscan = nc.vector.tensor_tensor_scan
                    ttscan(s_lr[:], at[:], xt[:], 0.0, MUL, ADD)
                    ttscan(s_rl[:], a_r, x_r, 0.0, MUL, ADD)
                    elem_eng.tensor_tensor(s_lr[:], s_lr[:], ilr[:], MUL)
                    elem_eng.tensor_tensor(s_rl[:], s_rl[:], irl[:], MUL)
                    m = work.tile([P, WD], f32, tag=f"m{pf}")
                    nc.vector.tensor_tensor(m[:], s_lr[:], _revap(s_rl[:]), ADD)
                    two_x = work.tile([P, WD], f32, tag=f"tx{pf}")
                    nc.scalar.mul(two_x[:], xt[:], 2.0)
                    nc.vector.copy_predicated(m[:], q[:], two_x[:])
                    return m

                def seg_sums_split(xt, at, cnts, pf, tail_eng=None):
                    """Critical-path variant: tail split into column halves —
                    half 0 (needed by the first store block) first; second
                    half's arithmetic goes to Pool to keep DVE clear.
                    Returns (m, finish_fn)."""
                    ilr, irl, q = cnts
                    s_lr = work.tile([P, WD], f32, tag=f"slr{pf}")
                    s_rl = work.tile([P, WD], f32, tag=f"srl{pf}")  # reversed
                    ttscan = nc.vector.tensor_tensor_scan
                    ttscan(s_lr[:], at[:], xt[:], 0.0, MUL, ADD)
                    ttscan(s_rl[:], _revap(at[:]), _revap(xt[:]), 0.0, MUL, ADD)
                    m = work.tile([P, WD], f32, tag=f"m{pf}")
                    two_x = work.tile([P, WD], f32, tag=f"tx{pf}")
                    nc.scalar.mul(two_x[:], xt[:], 2.0)
                    hd_ = WD // 2

                    def half(h, eng):
                        lo, hi = h * hd_, (h + 1) * hd_
                        rlo, rhi = WD - hi, WD - lo  # mirrored slice (rev space)
                        eng.tensor_tensor(
                            s_lr[:, lo:hi], s_lr[:, lo:hi], ilr[:, lo:hi], MUL)
                        eng.tensor_tensor(
                            s_rl[:, rlo:rhi], s_rl[:, rlo:rhi],
                            irl[:, rlo:rhi], MUL)
                        eng.tensor_tensor(
                            m[:, lo:hi], s_lr[:, lo:hi],
                            _revap(s_rl[:, rlo:rhi]), ADD)
                        nc.vector.copy_predicated(
                            m[:, lo:hi], q[:, lo:hi], two_x[:, lo:hi])

                    half(0, tail_eng or nc.vector)
                    return m, (lambda: half(1, nc.vector))

                def transpose_a(c):
                    """Transposed mask -> aT[c] (SBUF, bounced via PSUM:
                    scan data0 and data1 cannot both live in PSUM)."""
                    for t in range(NT):
                        pb = psum.tile([P, P], f32, tag="ptr")
                        nc.tensor.transpose(
                            pb[:], am[t][:, c * P:(c + 1) * P], ident[:]
                        )
                        nc.scalar.copy(aT[c][:, t * P:(t + 1) * P], pb[:])

                def transpose_x(c):
                    """Transpose x straight into the PSUM tile the vertical
                    sum-scans read."""
                    for t in range(NT):
                        nc.tensor.transpose(
                            xT[c][:, t * P:(t + 1) * P],
                            xdv(t, c * P, (c + 1) * P), ident[:],
                        )

                def combine_store(t, c, first=False):
                    """y block (t,c) = yh[t][:,cP:] + yv[c][:,tP:]^T;
                    expand 8x8, store 8 row-replicas. first=True keeps the
                    expansion on DVE (skips the ACT hop on the path that
                    opens the store stream)."""
                    pb = psum.tile([P, P], f32, tag="ptb")
                    nc.tensor.transpose(
                        pb[:], yv[c][:, t * P:(t + 1) * P], ident[:]
                    )
                    cw = W // NT
                    ye_view = ye[t][:, c * cw:(c + 1) * cw].rearrange(
                        "p (a b) -> p a b", b=8)
                    if first:
                        ysum = work.tile([P, P], f32, tag=f"ys{t}{c}")
                        nc.vector.tensor_tensor(
                            ysum[:], yh[t][:, c * P:(c + 1) * P], pb[:], ADD
                        )
                        nc.vector.tensor_copy(ye_view, _bcast8(ysum[:]))
                    else:
                        ysum = work.tile([P, P], f32, tag=f"ys{t}{c}")
                        nc.vector.tensor_tensor(
                            ysum[:], yh[t][:, c * P:(c + 1) * P], pb[:], ADD
                        )
                        nc.scalar.copy(ye_view, _bcast8(ysum[:]))
                    for k in range(8):
                        nc.sync.dma_start(
                            out=yg[t * P:(t + 1) * P, k, c * cw:(c + 1) * cw],
                            in_=ye[t][:, c * cw:(c + 1) * cw],
                        )

                # ---- ordered for earliest store start ----
                # masks first (x1 loads lead), count stages run during x0
                # loads, then sum stages feed the store stream
                load_x1(0)
                load_x1(1)
                load_x0_half(0, 0)
                load_x0_half(1, 0)
                load_x0_half(0, 1)
                load_x0_half(1, 1)
                ch0 = seg_counts(am[0], "h0")
                transpose_a(0)
                cv0 = seg_counts(aT[0], "v0")
                # the chain that opens the store stream gets top scheduling
                # priority so ready-but-noncritical work can't delay it
                with tc.high_priority():
                    transpose_x(0)
                    yv[0], fin_v0 = seg_sums_split(xT[0], aT[0], cv0, "v0")
                    yh[0], fin_h0 = seg_sums_split(xdv(0), am[0], ch0, "h0")
                    combine_store(0, 0, first=True)  # store stream opens
                fin_v0()
                fin_h0()
                ch1 = seg_counts(am[1], "h1")
                yh[1] = seg_sums(xdv(1), am[1], ch1, "h1")
                combine_store(1, 0)
                transpose_a(1)
                cv1 = seg_counts(aT[1], "v1")
                transpose_x(1)
                yv[1] = seg_sums(xT[1], aT[1], cv1, "v1")
                combine_store(0, 1)
                combine_store(1, 1)

            if loop_n > 1:
                with tc.For_i(0, loop_n, 1):
                    body()
            else:
                body()

    nc.compile()
    return nc


def _get_nc():
    if "nc" not in _CACHE:
        _CACHE["nc"] = build_program()
    return _CACHE["nc"]


def kernel(x0: np.ndarray, x1: np.ndarray) -> np.ndarray:
    from concourse.bass_utils import run_bass_kernel_spmd

    nc = _get_nc()
    n = x0.shape[0]
    in_maps = [
        {"x0": np.ascontiguousarray(x0[i, 0]),
         "x1": np.ascontiguousarray(x1[i, 0])}
        for i in range(n)
    ]
    res = run_bass_kernel_spmd(nc, in_maps, list(range(N_CORES)))
    out = np.stack([res.results[i]["y"] for i in range(n)])
    return out.reshape(n, 1, H, W).astype(np.float32)
